# revision 1
# baseline (speedup 1.0000x reference)
"""Trainium2 Bass kernel for nn_BitwiseTasNetRepeat (8 NeuronCores, SPMD).

Algorithm (validated in numpy golden model):
- Every BN feeds only sign(), so BN+sign collapses to a per-channel
  threshold compare against tau = mean - (beta/gamma)*sqrt(var+eps).
- Matmul operands are binary: s1/s3 in {0,1} with 2x-scaled sign weights
  (affine corrections folded into thresholds / final subtract).
- Conv input v = 2*swd1*[Y > tauY] in {0,+-2}; all per-channel offsets
  cancel by shift-invariance of the variances.
- Sharding: (4 sequences x 2 halves) = 8 shards of T with a 256-wide
  "big halo" that shrinks by 2^i per block (no halo exchange).
- Global BN stats via 3 tiny AllGathers per block + local rank-sum.
"""
import sys
sys.path.insert(0, '/opt/trn_rl_repo')
import numpy as np
import ml_dtypes

from concourse import bass, bacc, tile, mybir
from concourse.bass_utils import run_bass_kernel_spmd

F32 = mybir.dt.float32
BF16 = mybir.dt.bfloat16
AX = mybir.AxisListType
OP = mybir.AluOpType
AF = mybir.ActivationFunctionType

BLOCKS, B, H, T, KW, NSEQ = 8, 128, 512, 8000, 3, 4
EPS = 1e-5
NT = float(NSEQ * T)
NCORES = 8
TLOC = 4000
HALO = 256
W = TLOC + 2 * HALO            # 4512
O0, O1 = HALO, HALO + TLOC     # owned span [256, 4256)
HSEQ = [256 - (1 << i) for i in range(9)]
CH = 1024                      # psum tile free width (2 matmuls of 512)

# engine split knobs (True = ACT handles that group)
EVAC_ACT = [True, False, True, False]
Y2_ACT = [True, False, True, False]
Z2_ACT = [False, True, False, True]
PE_CONV = (0, 1, 2, 3)         # groups whose conv runs on PE (diag matmuls)
ZEV_ACT = [True, True, True, True]     # PE-conv z evac engine per group

_CACHED = {}


def ea(x):
    return x & ~1


def eb(x):
    return (x + 1) & ~1


def _chunks(lo, hi, step=CH):
    out = []
    c = lo
    while c < hi:
        out.append((c, min(c + step, hi)))
        c += step
    return out


def build_nc(emu_1core=False):
    ndev = 1 if emu_1core else NCORES
    nc = bacc.Bacc("TRN2", target_bir_lowering=False, debug=False,
                   num_devices=ndev)

    # ---------------- DRAM I/O ----------------
    xs_d = nc.dram_tensor("xs", [B, W], F32, kind="ExternalInput")
    mkL_d = nc.dram_tensor("mkL", [B, 256], BF16, kind="ExternalInput")
    mkiL_d = nc.dram_tensor("mkiL", [B, 256], BF16, kind="ExternalInput")
    mkR_d = nc.dram_tensor("mkR", [B, 256], BF16, kind="ExternalInput")
    mkiR_d = nc.dram_tensor("mkiR", [B, 256], BF16, kind="ExternalInput")
    w1e_d = nc.dram_tensor("w1e", [BLOCKS, B, H], BF16, kind="ExternalInput")
    w1f_d = nc.dram_tensor("w1f", [BLOCKS, B, H], F32, kind="ExternalInput")
    w2e_d = nc.dram_tensor("w2e", [BLOCKS, H, B], BF16, kind="ExternalInput")
    rho0_d = nc.dram_tensor("rho0", [BLOCKS, B, 4], F32, kind="ExternalInput")
    rho2_d = nc.dram_tensor("rho2", [BLOCKS, B, 4], F32, kind="ExternalInput")
    sd1_d = nc.dram_tensor("sd1", [BLOCKS, B, 4], F32, kind="ExternalInput")
    sd2_d = nc.dram_tensor("sd2", [BLOCKS, B, 4], F32, kind="ExternalInput")
    bg1_d = nc.dram_tensor("bg1", [BLOCKS, B, 1], F32, kind="ExternalInput")
    bg2_d = nc.dram_tensor("bg2", [BLOCKS, B, 4], F32, kind="ExternalInput")
    bg3_d = nc.dram_tensor("bg3", [BLOCKS, B, 4], F32, kind="ExternalInput")
    tau10_d = nc.dram_tensor("tau10", [B, 1], F32, kind="ExternalInput")
    dg01_d = nc.dram_tensor("dg01", [B, B], BF16, kind="ExternalInput")
    r8_d = nc.dram_tensor("r8", [B, 1], F32, kind="ExternalInput")
    out_d = nc.dram_tensor("out", [B, TLOC], F32, kind="ExternalOutput")

    with tile.TileContext(nc) as tc:
        with (
            tc.tile_pool(name="big", bufs=1) as bigp,
            tc.tile_pool(name="wt", bufs=1) as wtp,
            tc.tile_pool(name="sm", bufs=1) as smp,
            tc.tile_pool(name="blk", bufs=2) as blkp,
            tc.tile_pool(name="mmps", bufs=3, space="PSUM") as psp,
            tc.tile_pool(name="vps", bufs=2, space="PSUM") as vpsp,
            tc.tile_pool(name="dram", bufs=3, space="DRAM") as drp,
        ):
            # ------------- persistent SBUF -------------
            resid = bigp.tile([B, W], F32, tag="resid")
            s1e = bigp.tile([B, W], BF16, tag="s1e")       # DVE scratch
            sc2 = bigp.tile([B, W], BF16, tag="sc2")       # DVE scratch 2
            jact = bigp.tile([B, TLOC], BF16, tag="jact")  # ACT junk
            Ysb = [bigp.tile([B, W], BF16, tag=f"Y{g}", name=f"Ysb{g}")
                   for g in range(4)]
            sig = [bigp.tile([B, W], BF16, tag=f"S{g}", name=f"sig{g}")
                   for g in range(4)]
            w1sb = wtp.tile([B, BLOCKS * H], BF16, tag="w1sb")
            w2sb = wtp.tile([B, BLOCKS * H], BF16, tag="w2sb")
            mkLs = smp.tile([B, 256], BF16, tag="mkL")
            mkiLs = smp.tile([B, 256], BF16, tag="mkiL")
            mkRs = smp.tile([B, 256], BF16, tag="mkR")
            mkiRs = smp.tile([B, 256], BF16, tag="mkiR")
            rho0s = smp.tile([B, BLOCKS * 4], F32, tag="rho0")
            rho2s = smp.tile([B, BLOCKS * 4], F32, tag="rho2")
            sd1s = smp.tile([B, BLOCKS * 4], F32, tag="sd1")
            sd2s = smp.tile([B, BLOCKS * 4], F32, tag="sd2")
            bg1s = smp.tile([B, BLOCKS], F32, tag="bg1")
            bg2s = smp.tile([B, BLOCKS * 4], F32, tag="bg2")
            bg3s = smp.tile([B, BLOCKS * 4], F32, tag="bg3")
            r8s = smp.tile([B, 1], F32, tag="r8")
            tau1 = smp.tile([B, 1], F32, tag="tau1")
            dg01 = smp.tile([B, B], BF16, tag="dg01")

            # ------------- load constants -------------
            nc.sync.dma_start(resid[:], xs_d[:])
            for sb, dr in ((mkLs, mkL_d), (mkiLs, mkiL_d),
                           (mkRs, mkR_d), (mkiRs, mkiR_d)):
                nc.sync.dma_start(sb[:], dr[:])
            nc.sync.dma_start(w1sb[:].rearrange("p (i h) -> p i h", i=BLOCKS),
                              w1e_d.ap().rearrange("i c h -> c i h"))
            nc.sync.dma_start(
                w2sb[:].rearrange("p (i k o) -> p i k o", i=BLOCKS, k=4),
                w2e_d.ap().rearrange("i (k p) o -> p i k o", p=B))
            for sb, dr in ((rho0s, rho0_d), (rho2s, rho2_d), (sd1s, sd1_d),
                           (sd2s, sd2_d), (bg1s, bg1_d), (bg2s, bg2_d),
                           (bg3s, bg3_d)):
                nc.sync.dma_start(
                    sb[:].rearrange("p (i g) -> p i g", i=BLOCKS),
                    dr.ap().rearrange("i c g -> c i g"))
            nc.sync.dma_start(r8s[:], r8_d[:])
            nc.sync.dma_start(tau1[:], tau10_d[:])
            nc.sync.dma_start(dg01[:], dg01_d[:])

            def allgather(pay, kcols, name):
                """AG pay [B,kcols] across ranks, return rank-summed [B,kcols]."""
                bin_ = drp.tile([B, kcols], F32, tag=f"bin_{name}")
                bout = drp.tile([NCORES * B, kcols], F32, tag=f"bout_{name}")
                nc.sync.dma_start(bin_[:], pay[:])
                if emu_1core:
                    # approximate the ~4.6us AllGather floor with 3 DMAs
                    for r_ in range(3):
                        nc.sync.dma_start(bout[r_ * B:(r_ + 1) * B, :], bin_[:])
                else:
                    nc.gpsimd.collective_compute(
                        "AllGather", OP.bypass,
                        replica_groups=[list(range(NCORES))],
                        ins=[bin_.opt()], outs=[bout.opt()],
                    )
                gath = blkp.tile([B, kcols * NCORES], F32, tag=f"gath_{name}")
                nc.sync.dma_start(
                    gath[:].rearrange("p (k r) -> p k r", r=NCORES),
                    bout[:].rearrange("(r p) k -> p k r", r=NCORES, p=B))
                red = blkp.tile([B, kcols], F32, tag=f"red_{name}")
                nc.vector.tensor_reduce(
                    red[:],
                    gath[:].rearrange("p (k r) -> p k r", r=NCORES),
                    axis=AX.X, op=OP.add)
                return red

            def tau_from(mean_t, e2eps_t, bg_ap, out_ap, tag):
                """out = mean - bg*sqrt(e2eps - mean^2); e2eps includes +eps."""
                k = mean_t.shape[1]
                ve = blkp.tile([B, k], F32, tag=f"ve_{tag}")
                nc.vector.scalar_tensor_tensor(
                    ve[:], mean_t[:], -1.0, mean_t[:], OP.mult, OP.mult)
                nc.vector.tensor_tensor(ve[:], e2eps_t[:], ve[:], OP.add)
                s0 = blkp.tile([B, k], F32, tag=f"s0_{tag}")
                nc.scalar.activation(s0[:], ve[:], AF.Sqrt, bias=0.0, scale=1.0)
                rc = blkp.tile([B, k], F32, tag=f"rc_{tag}")
                nc.vector.reciprocal(rc[:], s0[:])
                # sq = 0.5*s0 + 0.5*ve*rc  (one Newton step)
                nc.vector.scalar_tensor_tensor(
                    rc[:], ve[:], 0.5, rc[:], OP.mult, OP.mult)
                nc.vector.scalar_tensor_tensor(
                    rc[:], s0[:], 0.5, rc[:], OP.mult, OP.add)
                nc.vector.tensor_tensor(rc[:], bg_ap, rc[:], OP.mult)
                nc.vector.tensor_tensor(out_ap, mean_t[:], rc[:], OP.subtract)

            for i in range(BLOCKS):
                d = 1 << i
                h_in, h_out = HSEQ[i], HSEQ[i + 1]
                A0, A1 = ea(O0 - h_in), eb(O1 + h_in)
                B0, B1 = ea(O0 - h_out), eb(O1 + h_out)
                assert A0 <= B0 - d and B1 + d <= A1 and A0 >= 0 and A1 <= W

                payA1 = blkp.tile([B, 3], F32, tag="payA1")
                payA2 = blkp.tile([B, 2], F32, tag="payA2")
                payB1 = blkp.tile([B, 4], F32, tag="payB1")
                payB2 = blkp.tile([B, 4], F32, tag="payB2")
                payC = blkp.tile([B, 2], F32, tag="payC")
                w1ft = blkp.tile([B, H], F32, tag="w1ft")
                nc.sync.dma_start(w1ft[:], w1f_d[i])

                # ---- s1 = [resid > tau1] in {0,1} ----
                nc.vector.tensor_scalar(
                    s1e[:, A0:O0], resid[:, A0:O0], tau1[:, 0:1], None, OP.is_gt)
                nc.vector.tensor_scalar(
                    s1e[:, O0:O1], resid[:, O0:O1], tau1[:, 0:1], 0.0,
                    OP.is_gt, OP.add, accum_out=payA1[:, 0:1])
                nc.vector.tensor_scalar(
                    s1e[:, O1:A1], resid[:, O1:A1], tau1[:, 0:1], None, OP.is_gt)

                def mm1_group(g):
                    for (c0, c1) in _chunks(A0, A1):
                        ps = psp.tile([B, CH], F32, tag="mmps", name="ps_mm1")
                        for (s0_, s1_) in _chunks(c0, c1, 512):
                            nc.tensor.matmul(
                                ps[:, s0_ - c0:s1_ - c0],
                                w1sb[:, i * H + g * B: i * H + (g + 1) * B],
                                s1e[:, s0_:s1_], start=True, stop=True)
                        if EVAC_ACT[g]:
                            nc.scalar.copy(Ysb[g][:, c0:c1], ps[:, 0:c1 - c0])
                        else:
                            nc.vector.tensor_copy(Ysb[g][:, c0:c1],
                                                  ps[:, 0:c1 - c0])

                def sumY2_group(g, acc_ap):
                    if Y2_ACT[g]:
                        nc.scalar.activation(
                            jact[:], Ysb[g][:, O0:O1], AF.Square,
                            bias=0.0, scale=1.0, accum_out=acc_ap)
                    else:
                        nc.vector.tensor_tensor(
                            sc2[:, O0:O1], Ysb[g][:, O0:O1], Ysb[g][:, O0:O1],
                            OP.mult)
                        nc.vector.tensor_scalar(
                            sc2[:, O0:O1], sc2[:, O0:O1], 1.0, 0.0,
                            OP.mult, OP.add, accum_out=acc_ap)

                for g in (0, 1):
                    mm1_group(g)
                    sumY2_group(g, payA1[:, 1 + g:2 + g])
                redA1 = allgather(payA1, 3, "A1")
                for g in (2, 3):
                    mm1_group(g)
                    sumY2_group(g, payA2[:, g - 2:g - 1])
                redA2 = allgather(payA2, 2, "A2")

                # E[Y] matvec for all groups (needs global Ss1 from A1)
                sumY = blkp.tile([B, 4], F32, tag="sumY")
                for g in range(4):
                    pv = vpsp.tile([B, 1], F32, tag="vps", name="pv_mv")
                    nc.tensor.matmul(pv[:], w1ft[:, g * B:(g + 1) * B],
                                     redA1[:, 0:1], start=True, stop=True)
                    nc.vector.tensor_copy(sumY[:, g:g + 1], pv[:])
                EY = blkp.tile([B, 4], F32, tag="EY")
                nc.vector.tensor_scalar(EY[:], sumY[:], 1.0 / NT, None, OP.mult)

                tauY = blkp.tile([B, 4], F32, tag="tauY")
                EY2e = blkp.tile([B, 4], F32, tag="EY2e")

                def tau2_half(h01, red, off):
                    g0 = 0 if h01 else 2
                    sl = slice(g0, g0 + 2)
                    nc.vector.tensor_scalar(
                        EY2e[:, sl], red[:, off:off + 2], 1.0 / NT, EPS,
                        OP.mult, OP.add)
                    tau_from(EY[:, sl], EY2e[:, sl],
                             bg2s[:, i * 4 + g0:i * 4 + g0 + 2],
                             tauY[:, sl], f"t2{g0}")

                def sigv_group(g):
                    sd1c = sd1s[:, i * 4 + g:i * 4 + g + 1]
                    nc.vector.tensor_scalar(
                        sig[g][:, A0:A1], Ysb[g][:, A0:A1],
                        tauY[:, g:g + 1], sd2s[:, i * 4 + g:i * 4 + g + 1],
                        OP.is_gt, OP.mult)
                    wl = O0 - A0
                    nc.vector.tensor_tensor(
                        sig[g][:, A0:O0], sig[g][:, A0:O0], mkLs[:, 0:wl],
                        OP.mult)
                    nc.vector.scalar_tensor_tensor(
                        sig[g][:, A0:O0], mkiLs[:, 0:wl], sd1c,
                        sig[g][:, A0:O0], OP.mult, OP.add)
                    wr = A1 - O1
                    nc.vector.tensor_tensor(
                        sig[g][:, O1:A1], sig[g][:, O1:A1], mkRs[:, 0:wr],
                        OP.mult)
                    nc.vector.scalar_tensor_tensor(
                        sig[g][:, O1:A1], mkiRs[:, 0:wr], sd1c,
                        sig[g][:, O1:A1], OP.mult, OP.add)

                dgs = {}
                for g in PE_CONV:
                    dg0 = blkp.tile([B, B], BF16, tag=f"dg0_{g}", name=f"dg0_{g}")
                    dg2 = blkp.tile([B, B], BF16, tag=f"dg2_{g}", name=f"dg2_{g}")
                    nc.vector.tensor_scalar(
                        dg0[:], dg01[:], rho0s[:, i * 4 + g:i * 4 + g + 1],
                        None, OP.mult)
                    nc.vector.tensor_scalar(
                        dg2[:], dg01[:], rho2s[:, i * 4 + g:i * 4 + g + 1],
                        None, OP.mult)
                    dgs[g] = (dg0, dg2)

                def conv_group(g):
                    z = Ysb[g]
                    if g in PE_CONV:
                        dg0, dg2 = dgs[g]
                        for (c0, c1) in _chunks(B0, B1):
                            ps = psp.tile([B, CH], F32, tag="mmps", name="ps_cv")
                            for (s0_, s1_) in _chunks(c0, c1, 512):
                                nc.tensor.matmul(
                                    ps[:, s0_ - c0:s1_ - c0], dg0[:],
                                    sig[g][:, s0_ - d:s1_ - d],
                                    start=True, stop=False)
                                nc.tensor.matmul(
                                    ps[:, s0_ - c0:s1_ - c0], dg01[:],
                                    sig[g][:, s0_:s1_],
                                    start=False, stop=False)
                                nc.tensor.matmul(
                                    ps[:, s0_ - c0:s1_ - c0], dg2[:],
                                    sig[g][:, s0_ + d:s1_ + d],
                                    start=False, stop=True)
                            if ZEV_ACT[g]:
                                nc.scalar.copy(z[:, c0:c1], ps[:, 0:c1 - c0])
                            else:
                                nc.vector.tensor_copy(z[:, c0:c1],
                                                      ps[:, 0:c1 - c0])
                    else:
                        nc.vector.tensor_scalar(
                            s1e[:, B0 - d:B1 - d], sig[g][:, B0 - d:B1 - d],
                            rho0s[:, i * 4 + g:i * 4 + g + 1], None, OP.mult)
                        nc.vector.tensor_tensor(
                            sc2[:, B0:B1], s1e[:, B0 - d:B1 - d],
                            sig[g][:, B0:B1], OP.add)
                        nc.vector.tensor_scalar(
                            s1e[:, B0 + d:B1 + d], sig[g][:, B0 + d:B1 + d],
                            rho2s[:, i * 4 + g:i * 4 + g + 1], None, OP.mult)
                        nc.vector.tensor_tensor(
                            z[:, B0:B1], sc2[:, B0:B1], s1e[:, B0 + d:B1 + d],
                            OP.add)

                def zstats_group(g, pay, col):
                    z = Ysb[g]
                    nc.vector.tensor_scalar(
                        s1e[:, O0:O1], z[:, O0:O1], 1.0, 0.0,
                        OP.mult, OP.add, accum_out=pay[:, col:col + 1])
                    if Z2_ACT[g]:
                        nc.scalar.activation(
                            jact[:], z[:, O0:O1], AF.Square,
                            bias=0.0, scale=1.0, accum_out=pay[:, col + 2:col + 3])
                    else:
                        nc.vector.tensor_tensor(
                            sc2[:, O0:O1], z[:, O0:O1], z[:, O0:O1], OP.mult)
                        nc.vector.tensor_scalar(
                            sc2[:, O0:O1], sc2[:, O0:O1], 1.0, 0.0,
                            OP.mult, OP.add, accum_out=pay[:, col + 2:col + 3])

                # half 1: groups 0,1 on DVE conv
                tau2_half(True, redA1, 1)
                for g in (0, 1):
                    sigv_group(g)
                    conv_group(g)
                    zstats_group(g, payB1, g)
                redB1 = allgather(payB1, 4, "B1")

                # half 2: groups 2,3 (PE conv)
                tau2_half(False, redA2, 0)
                for g in (2, 3):
                    sigv_group(g)
                    conv_group(g)
                    zstats_group(g, payB2, g - 2)
                redB2 = allgather(payB2, 4, "B2")

                # ---- tau3c halves + s3 ----
                tau3c = blkp.tile([B, 4], F32, tag="tau3c")
                mh = blkp.tile([B, 4], F32, tag="mh")
                eh2e = blkp.tile([B, 4], F32, tag="eh2e")

                def tau3_half(h01, red):
                    g0 = 0 if h01 else 2
                    sl = slice(g0, g0 + 2)
                    nc.vector.tensor_scalar(mh[:, sl], red[:, 0:2], 1.0 / NT,
                                            None, OP.mult)
                    nc.vector.tensor_scalar(
                        eh2e[:, sl], red[:, 2:4], 1.0 / NT, EPS, OP.mult, OP.add)
                    tau_from(mh[:, sl], eh2e[:, sl],
                             bg3s[:, i * 4 + g0:i * 4 + g0 + 2],
                             tau3c[:, sl], f"t3{g0}")

                tau3_half(True, redB1)
                for g in (0, 1):
                    nc.vector.tensor_scalar(
                        sig[g][:, B0:B1], Ysb[g][:, B0:B1],
                        tau3c[:, g:g + 1], None, OP.is_gt)
                tau3_half(False, redB2)
                for g in (2, 3):
                    nc.vector.tensor_scalar(
                        sig[g][:, B0:B1], Ysb[g][:, B0:B1],
                        tau3c[:, g:g + 1], None, OP.is_gt)

                # ---- mm2 + resid update (+ resid^2 accum per sub-span) ----
                def subspans(c0, c1):
                    subs = []
                    s_lo = c0
                    for edge in (O0, O1):
                        if s_lo < edge < c1:
                            subs.append((s_lo, edge))
                            s_lo = edge
                    subs.append((s_lo, c1))
                    return subs

                nacc = sum(1 for (c0, c1) in _chunks(B0, B1)
                           for (s0_, s1_) in subspans(c0, c1)
                           if s0_ >= O0 and s1_ <= O1)
                accR = blkp.tile([B, nacc], F32, tag="accR")
                accR2 = blkp.tile([B, nacc], F32, tag="accR2")
                iacc = 0
                for (c0, c1) in _chunks(B0, B1):
                    ps = psp.tile([B, CH], F32, tag="mmps", name="ps_mm2")
                    for kg in range(4):
                        for (s0_, s1_) in _chunks(c0, c1, 512):
                            nc.tensor.matmul(
                                ps[:, s0_ - c0:s1_ - c0],
                                w2sb[:, i * H + kg * B: i * H + (kg + 1) * B],
                                sig[kg][:, s0_:s1_],
                                start=(kg == 0), stop=(kg == 3))
                    for (s0_, s1_) in subspans(c0, c1):
                        owned = s0_ >= O0 and s1_ <= O1
                        if owned:
                            nc.vector.scalar_tensor_tensor(
                                resid[:, s0_:s1_], ps[:, s0_ - c0:s1_ - c0], 1.0,
                                resid[:, s0_:s1_], OP.mult, OP.add,
                                accum_out=accR[:, iacc:iacc + 1])
                            nc.scalar.activation(
                                jact[:, 0:s1_ - s0_], resid[:, s0_:s1_],
                                AF.Square, bias=0.0, scale=1.0,
                                accum_out=accR2[:, iacc:iacc + 1])
                            iacc += 1
                        else:
                            nc.vector.scalar_tensor_tensor(
                                resid[:, s0_:s1_], ps[:, s0_ - c0:s1_ - c0], 1.0,
                                resid[:, s0_:s1_], OP.mult, OP.add)
                nc.vector.tensor_reduce(payC[:, 0:1], accR[:], axis=AX.X,
                                        op=OP.add)
                nc.vector.tensor_reduce(payC[:, 1:2], accR2[:], axis=AX.X,
                                        op=OP.add)

                # ---- AG_C -> tau1 for next block ----
                if i < BLOCKS - 1:
                    redC = allgather(payC, 2, "Cp")
                    mS = blkp.tile([B, 1], F32, tag="mS")
                    nc.vector.tensor_scalar(mS[:], redC[:, 0:1], 1.0 / NT, None,
                                            OP.mult)
                    er2e = blkp.tile([B, 1], F32, tag="er2e")
                    nc.vector.tensor_scalar(
                        er2e[:], redC[:, 1:2], 1.0 / NT, EPS, OP.mult, OP.add)
                    tau_from(mS, er2e, bg1s[:, i + 1:i + 2], tau1[:, 0:1], "t1")

            # ---- output ----
            outb = bigp.tile([B, TLOC], F32, tag="outb")
            nc.vector.tensor_scalar(
                outb[:], resid[:, O0:O1], r8s[:, 0:1], None, OP.subtract)
            nc.sync.dma_start(out_d[:], outb[:])

    nc.compile()
    return nc


def _prep(x, W1, Wd, W2, g1, b1, g2, b2, g3, b3):
    bf = ml_dtypes.bfloat16
    sw1 = np.sign(W1).astype(np.float32)
    swd = np.sign(Wd).astype(np.float32)
    sw2 = np.sign(W2).astype(np.float32)
    w1e = (2.0 * sw1).transpose(0, 2, 1).copy()          # [8,128,512] (c,o)
    w2e = (2.0 * sw2).transpose(0, 2, 1).copy()          # [8,512,128] (h,o)
    sd1 = swd[:, :, 1]

    def pack(a):  # [8,512] -> [8,128,4]
        return a.reshape(BLOCKS, 4, B).transpose(0, 2, 1).copy().astype(np.float32)

    m = x.mean(axis=(0, 2)); v = x.var(axis=(0, 2))
    bg1 = (b1 / g1)[:, :, None].astype(np.float32)
    tau10 = (m - bg1[0, :, 0] * np.sqrt(v + EPS))[:, None].astype(np.float32)
    r8 = sw2.sum(axis=2).sum(axis=0)[:, None].astype(np.float32)

    shared = {
        "w1e": w1e.astype(bf), "w1f": w1e.astype(np.float32),
        "w2e": w2e.astype(bf),
        "rho0": pack(swd[:, :, 0] * sd1), "rho2": pack(swd[:, :, 2] * sd1),
        "sd1": pack(sd1), "sd2": pack(2.0 * sd1),
        "bg1": bg1, "bg2": pack(b2 / g2), "bg3": pack(b3 / g3),
        "tau10": tau10, "r8": r8,
        "dg01": np.eye(B, dtype=bf),
    }
    in_maps = []
    ones = np.ones((B, 256), bf)
    zeros = np.zeros((B, 256), bf)
    for r in range(NCORES):
        seq, half = r // 2, r % 2
        t0 = half * TLOC
        lo, hi = t0 - HALO, t0 + TLOC + HALO
        xs = np.zeros((B, W), np.float32)
        slo, shi = max(lo, 0), min(hi, T)
        xs[:, slo - lo:shi - lo] = x[seq, :, slo:shi]
        m_ = dict(shared)
        m_["xs"] = xs
        # left halo (cols < O0) is out-of-seq iff half==0; right iff half==1.
        # masked side: v' = v*0 + sd1*1 = sd1 (the s2=0 zero-pad equivalent);
        # interior side: v' = v*1 + sd1*0 = v.
        m_["mkL"] = zeros if half == 0 else ones
        m_["mkiL"] = ones if half == 0 else zeros
        m_["mkR"] = ones if half == 0 else zeros
        m_["mkiR"] = zeros if half == 0 else ones
        in_maps.append(m_)
    return in_maps


def kernel(**inputs):
    inputs = {k: np.asarray(v, dtype=np.float32) for k, v in inputs.items()}
    if "nc" not in _CACHED:
        _CACHED["nc"] = build_nc()
    nc = _CACHED["nc"]
    in_maps = _prep(**inputs)
    res = run_bass_kernel_spmd(nc, in_maps, core_ids=list(range(NCORES)))
    out = np.zeros((NSEQ, B, T), np.float32)
    for r in range(NCORES):
        seq, half = r // 2, r % 2
        out[seq, :, half * TLOC:(half + 1) * TLOC] = res.results[r]["out"]
    return out

